# revision 1
# baseline (speedup 1.0000x reference)
"""EncDecAD (LSTM encoder-decoder) Trainium2 kernel, 8 NeuronCores.

Strategy: tensor-parallel over the 4*HID=4096 gate columns (each core owns a
128-column slice of each gate i/j/f/o => 512 local gate cols). Per recurrent
step each core computes its z-slice [B=128, 512] (8 accumulating matmuls with
the gathered transposed hidden state as the stationary operand), applies the
gate nonlinearities for its HID/8 slice of (c, h), transposes h (and c on
output-producing steps) back to hid-major, and AllGathers the slices so every
core has the full h_T for the next step. The x-dependent halves of the gate
pre-activations are precomputed in hardware loops (fully parallel over S*B
rows): the encoder x-part inline per step, the decoder input transform
tanh(x@W_s+b_s)@W_dec_x materialized to DRAM. All matmul operands are bf16
(fp32 PSUM accumulation, fp32 cell state); measured end-to-end scale-relative
error ~8e-3.

Collectives cannot live inside hardware loops (per-instruction pre-staged
descriptor rings), so the 2047 recurrent steps are fully unrolled.
"""
import sys
import numpy as np

try:
    import concourse.bass as bass
except ImportError:
    sys.path.insert(0, "/opt/trn_rl_repo")
    import concourse.bass as bass

import ml_dtypes
import concourse.tile as tile
from concourse import bacc, mybir
from concourse.bass import ds
from concourse.bass_utils import run_bass_kernel_spmd
from concourse.masks import make_identity

BF16 = mybir.dt.bfloat16
F32 = mybir.dt.float32
NPBF16 = ml_dtypes.bfloat16

S, B, OBS, HID = 1024, 128, 64, 1024
NCORES = 8
GL = 4 * HID // NCORES          # local gate cols = 512
HL = HID // NCORES              # local hid slice = 128
KT = HID // 128                 # k-tiles over hid = 8
FORGET_BIAS = 1.0
ACT = mybir.ActivationFunctionType


def _core_cols(c):
    # reference gate order in W: [i | j | f | o]; local layout [i | o | f | j]
    i0, j0, f0, o0 = 0 * HID, 1 * HID, 2 * HID, 3 * HID
    s = c * HL
    return np.r_[i0 + s:i0 + s + HL, o0 + s:o0 + s + HL,
                 f0 + s:f0 + s + HL, j0 + s:j0 + s + HL]


def build_program(enc_steps=S, dec_steps=S - 1, n_cores=NCORES, interleave=True):
    """Build the SPMD Bass program. Returns finalized nc.

    interleave=True emits one decoder-precompute row-block inside each encoder
    step (PE work lands in the AllGather wait window, keeping the PE busy and
    the HAM clock-gate warm); False uses a separate For_i hardware loop before
    the recurrence."""
    nc = bacc.Bacc("TRN2", target_bir_lowering=False, debug=False,
                   num_devices=n_cores)
    core_ids = list(range(n_cores))
    NROWS = S * B  # 131072 flattened (t, b) rows

    # ---- I/O ----
    xT = nc.declare_dram_parameter("xT", [OBS + 1, NROWS], BF16, isOutput=False)
    wex = nc.declare_dram_parameter("wex", [OBS + 1, GL], BF16, isOutput=False)
    wsb = nc.declare_dram_parameter("wsb", [OBS + 1, HID], BF16, isOutput=False)
    wdx = nc.declare_dram_parameter("wdx", [HID, GL], BF16, isOutput=False)
    weh = nc.declare_dram_parameter("weh", [HID, GL], BF16, isOutput=False)
    wdh = nc.declare_dram_parameter("wdh", [HID, GL], BF16, isOutput=False)
    wo = nc.declare_dram_parameter("wo", [HID, OBS], BF16, isOutput=False)
    wob = nc.declare_dram_parameter("wob", [1, OBS], BF16, isOutput=False)
    y = nc.declare_dram_parameter("y", [NROWS, OBS], F32, isOutput=True)

    # ---- internal DRAM ----
    dpre = nc.dram_tensor("dpre", [NROWS, GL], BF16)  # decoder x-part, rows (t,b)
    # gather bounce buffers: h-only (encoder steps) and h+c (y-producing steps),
    # each double-buffered so consecutive collectives don't share a buffer
    gin_h = [nc.dram_tensor(f"ginh{p}", [128, HL], BF16) for p in range(2)]
    gout_h = [nc.dram_tensor(f"gouth{p}", [HID, HL], BF16, addr_space="Shared")
              for p in range(2)]
    gin_hc = [nc.dram_tensor(f"ginhc{p}", [128, 2 * HL], BF16) for p in range(2)]
    gout_hc = [nc.dram_tensor(f"gouthc{p}", [HID, 2 * HL], BF16, addr_space="Shared")
               for p in range(2)]

    with tile.TileContext(nc) as tc:
        with tc.tile_pool(name="wpool", bufs=1) as wp, \
             tc.tile_pool(name="state", bufs=1) as st, \
             tc.tile_pool(name="sb", bufs=3) as sb, \
             tc.tile_pool(name="pre", bufs=4) as prep, \
             tc.tile_pool(name="ps_z", bufs=2, space="PSUM") as ps_z, \
             tc.tile_pool(name="ps_tp", bufs=2, space="PSUM") as ps_tp, \
             tc.tile_pool(name="ps_s", bufs=1, space="PSUM") as ps_s, \
             tc.tile_pool(name="ps_pp", bufs=2, space="PSUM") as ps_pp:

            # ---- persistent SBUF ----
            weh_sb = wp.tile([128, KT * GL], BF16, tag="weh")
            wdh_sb = wp.tile([128, KT * GL], BF16, tag="wdh")
            wdx_sb = wp.tile([128, KT * GL], BF16, tag="wdx")
            wex_sb = wp.tile([OBS + 1, GL], BF16, tag="wex")
            wsb_sb = wp.tile([OBS + 1, HID], BF16, tag="wsb")
            wo_sb = wp.tile([128, KT * OBS], BF16, tag="wo")
            wob_sb = wp.tile([1, OBS], BF16, tag="wob")
            ones_sb = wp.tile([1, 128], BF16, tag="ones")
            ident = wp.tile([128, 128], BF16, tag="ident")
            hT_sb = st.tile([128, KT * 128], BF16, tag="hT")   # gathered h_T
            cT_sb = st.tile([128, KT * 128], BF16, tag="cT")   # gathered c_T
            c_sb = st.tile([128, HL], F32, tag="c")            # local cell state

            def ld(dst, src_kpn):
                nc.sync.dma_start(dst, src_kpn)

            ld(weh_sb[:].rearrange("p (k n) -> p k n", k=KT),
               weh.rearrange("(k p) n -> p k n", p=128))
            ld(wdh_sb[:].rearrange("p (k n) -> p k n", k=KT),
               wdh.rearrange("(k p) n -> p k n", p=128))
            ld(wdx_sb[:].rearrange("p (k n) -> p k n", k=KT),
               wdx.rearrange("(k p) n -> p k n", p=128))
            ld(wex_sb[:], wex[:, :])
            ld(wsb_sb[:], wsb[:, :])
            ld(wo_sb[:].rearrange("p (k n) -> p k n", k=KT),
               wo.rearrange("(k p) n -> p k n", p=128))
            ld(wob_sb[:], wob[:, :])
            nc.vector.memset(ones_sb[:], 1.0)
            make_identity(nc, ident[:])
            nc.vector.memset(hT_sb[:], 0.0)
            nc.vector.memset(c_sb[:], 0.0)

            # ---- precompute: dpre[r] = tanh([x,1]@[Ws;bs]) @ Wdx  (hw loop) ----
            NBLK = NROWS // 128  # 1024 row-blocks
            def pre_body(i):
                xt = sb.tile([OBS + 1, 128], BF16, tag="xt")
                src = xT[:, ds(i * 128, 128)] if not isinstance(i, int) \
                    else xT[:, i * 128:(i + 1) * 128]
                nc.sync.dma_start(xt[:], src)
                s_ps = ps_s.tile([128, HID], F32, tag="s")
                nc.tensor.matmul(s_ps[:, 0:512], xt[:], wsb_sb[:, 0:512],
                                 start=True, stop=True)
                nc.tensor.matmul(s_ps[:, 512:1024], xt[:], wsb_sb[:, 512:1024],
                                 start=True, stop=True)
                th = sb.tile([128, HID], BF16, tag="th")
                nc.scalar.activation(th[:], s_ps[:], ACT.Tanh)
                itp = sb.tile([128, HID], BF16, tag="itp")
                for k in range(KT):
                    tp = ps_tp.tile([128, 128], BF16, tag="tp")
                    nc.tensor.transpose(tp[:], th[:, k * 128:(k + 1) * 128], ident[:])
                    dst = itp[:, k * 128:(k + 1) * 128]
                    if k % 2 == 0:
                        nc.scalar.copy(dst, tp[:])
                    else:
                        nc.vector.tensor_copy(dst, tp[:])
                dp = ps_pp.tile([128, GL], F32, tag="pp")
                for k in range(KT):
                    nc.tensor.matmul(dp[:], itp[:, k * 128:(k + 1) * 128],
                                     wdx_sb[:, k * GL:(k + 1) * GL],
                                     start=(k == 0), stop=(k == KT - 1))
                dpsb = sb.tile([128, GL], BF16, tag="dpsb")
                nc.vector.tensor_copy(dpsb[:], dp[:])
                dst = dpre[ds(i * 128, 128), :] if not isinstance(i, int) \
                    else dpre[i * 128:(i + 1) * 128, :]
                nc.sync.dma_start(dst, dpsb[:])

            if not interleave:
                with tc.For_i(0, NBLK) as i:
                    pre_body(i)

            # ---- recurrent step ----
            def step(phase, t, t_out):
                """phase: 'enc'|'dec'. t: step index in phase. t_out: y row-block
                index or None."""
                par = (t if phase == "enc" else t + enc_steps) % 2
                w_sb = weh_sb if phase == "enc" else wdh_sb
                zp = ps_z.tile([128, GL], F32, tag="z")
                if phase == "enc":
                    # x-part inline: rows (t*B .. t*B+128)
                    xt = sb.tile([OBS + 1, 128], BF16, tag="ext")
                    nc.sync.dma_start(xt[:], xT[:, t * B:t * B + 128])
                    nc.tensor.matmul(zp[:], xt[:], wex_sb[:], start=True, stop=False)
                else:
                    pr = prep.tile([128, GL], BF16, tag="pr")
                    # decoder consumes x_{S-1-t}: rows ((S-1-t)*B ...)
                    nc.sync.dma_start(pr[:], dpre[(S - 1 - t) * B:(S - 1 - t) * B + 128, :])
                for k in range(KT):
                    nc.tensor.matmul(zp[:], hT_sb[:, k * 128:(k + 1) * 128],
                                     w_sb[:, k * GL:(k + 1) * GL],
                                     start=(phase == "dec" and k == 0),
                                     stop=(k == KT - 1))
                if phase == "dec":
                    zf = sb.tile([128, GL], F32, tag="zf")
                    nc.vector.tensor_add(zf[:], zp[:], pr[:])
                    zv = zf
                else:
                    zv = zp
                # gates: local cols [i | o | f | j]
                sio = sb.tile([128, 2 * HL], F32, tag="sio")
                nc.scalar.activation(sio[:], zv[:, 0:2 * HL], ACT.Sigmoid)
                sf = sb.tile([128, HL], F32, tag="sf")
                nc.scalar.activation(sf[:], zv[:, 2 * HL:3 * HL], ACT.Sigmoid,
                                     bias=FORGET_BIAS)
                tj = sb.tile([128, HL], F32, tag="tj")
                nc.scalar.activation(tj[:], zv[:, 3 * HL:4 * HL], ACT.Tanh)
                t1 = sb.tile([128, HL], F32, tag="t1")
                nc.vector.tensor_mul(t1[:], sio[:, 0:HL], tj[:])
                nc.vector.tensor_mul(c_sb[:], c_sb[:], sf[:])
                nc.vector.tensor_add(c_sb[:], c_sb[:], t1[:])
                tch = sb.tile([128, HL], F32, tag="tch")
                nc.scalar.activation(tch[:], c_sb[:], ACT.Tanh)
                hb = sb.tile([128, HL], BF16, tag="hb")
                nc.vector.tensor_mul(hb[:], sio[:, HL:2 * HL], tch[:])
                # transpose h (and c when needed) -> [hid_slice, batch]
                gsb = sb.tile([128, 2 * HL], BF16, tag="gsb")
                tp = ps_tp.tile([128, 128], BF16, tag="tp")
                nc.tensor.transpose(tp[:], hb[:], ident[:])
                nc.scalar.copy(gsb[:, 0:HL], tp[:])
                gather_c = t_out is not None or (phase == "enc" and t == enc_steps - 1)
                if gather_c:
                    cb = sb.tile([128, HL], BF16, tag="cb")
                    nc.vector.tensor_copy(cb[:], c_sb[:])
                    tp2 = ps_tp.tile([128, 128], BF16, tag="tp")
                    nc.tensor.transpose(tp2[:], cb[:], ident[:])
                    nc.vector.tensor_copy(gsb[:, HL:2 * HL], tp2[:])
                width = 2 * HL if gather_c else HL
                gi = (gin_hc if gather_c else gin_h)[par]
                go = (gout_hc if gather_c else gout_h)[par]
                nc.sync.dma_start(gi[:, :], gsb[:, 0:width])
                nc.gpsimd.collective_compute(
                    "AllGather", mybir.AluOpType.bypass,
                    replica_groups=[core_ids],
                    ins=[gi[:, :]],
                    outs=[go[:, :]],
                )
                # scatter gathered h_T into SBUF (8 block DMAs for queue parallelism)
                for k in range(KT):
                    nc.sync.dma_start(hT_sb[:, k * 128:(k + 1) * 128],
                                      go[k * 128:(k + 1) * 128, 0:HL])
                if gather_c:
                    nc.sync.dma_start(
                        cT_sb[:].rearrange("p (k b) -> p k b", k=KT),
                        go[:, HL:2 * HL].rearrange("(k p) b -> p k b", p=128))
                    # y = c @ Wo + bo
                    yp = ps_tp.tile([128, OBS], F32, tag="tp")
                    for k in range(KT):
                        nc.tensor.matmul(yp[:], cT_sb[:, k * 128:(k + 1) * 128],
                                         wo_sb[:, k * OBS:(k + 1) * OBS],
                                         start=(k == 0), stop=False)
                    nc.tensor.matmul(yp[:], ones_sb[:], wob_sb[:],
                                     start=False, stop=True)
                    ysb = sb.tile([128, OBS], F32, tag="ysb")
                    nc.scalar.copy(ysb[:], yp[:])
                    nc.sync.dma_start(y[t_out * B:t_out * B + 128, :], ysb[:])

            blocks_per_step = -(-NBLK // max(enc_steps, 1))  # ceil
            nxt = 0
            for t in range(enc_steps):
                # encoder final step gathers c and emits out_init at y[S-1]
                step("enc", t, S - 1 if t == enc_steps - 1 else None)
                if interleave:
                    for _ in range(blocks_per_step):
                        if nxt < NBLK:
                            pre_body(nxt)
                            nxt += 1
            for t in range(dec_steps):
                # decoder step t consumes x_{S-1-t}, emits y[S-2-t]
                step("dec", t, S - 2 - t)

    nc.finalize()
    return nc


def make_in_maps(inputs, n_cores=NCORES):
    X = np.asarray(inputs["input_seq"], np.float32)          # [S, B, OBS]
    W_enc = np.asarray(inputs["W_enc"], np.float32)
    b_enc = np.asarray(inputs["b_enc"], np.float32)
    W_dec = np.asarray(inputs["W_dec"], np.float32)
    b_dec = np.asarray(inputs["b_dec"], np.float32)
    W_s = np.asarray(inputs["W_s"], np.float32)
    b_s = np.asarray(inputs["b_s"], np.float32)
    W_o = np.asarray(inputs["W_o"], np.float32)
    b_o = np.asarray(inputs["b_o"], np.float32)

    We_x, We_h = W_enc[:OBS], W_enc[OBS:]
    Wd_x, Wd_h = W_dec[:HID], W_dec[HID:]
    Xf = X.reshape(S * B, OBS)
    xT = np.empty((OBS + 1, S * B), NPBF16)
    xT[:OBS] = Xf.T.astype(NPBF16)
    xT[OBS] = NPBF16(1.0)
    wsb = np.concatenate([W_s, b_s[None, :]], 0).astype(NPBF16)

    in_maps = []
    for c in range(n_cores):
        cols = _core_cols(c)
        wex = np.concatenate([We_x[:, cols], b_enc[cols][None, :]], 0).astype(NPBF16)
        m = {
            "xT": xT,
            "wex": wex,
            "wsb": wsb,
            "wdx": (Wd_x[:, cols] + 0.0).astype(NPBF16),
            "weh": (We_h[:, cols] + 0.0).astype(NPBF16),
            "wdh": (Wd_h[:, cols] + 0.0).astype(NPBF16),
            "wo": W_o.astype(NPBF16),
            "wob": b_o[None, :].astype(NPBF16),
        }
        # note: b_dec is folded nowhere; it is zero in this problem. Guard:
        assert np.abs(b_dec).max() == 0.0, "b_dec folding not implemented"
        in_maps.append(m)
    return in_maps


_PROGRAM_CACHE = {}


def kernel(**inputs) -> np.ndarray:
    key = ("full", NCORES)
    if key not in _PROGRAM_CACHE:
        _PROGRAM_CACHE[key] = build_program()
    nc = _PROGRAM_CACHE[key]
    in_maps = make_in_maps(inputs)
    res = run_bass_kernel_spmd(nc, in_maps, list(range(NCORES)))
    yout = res.results[0]["y"]  # identical on every core
    return np.ascontiguousarray(yout.reshape(S, B, OBS).astype(np.float32))


if __name__ == "__main__":
    import time
    t0 = time.time()
    nc = build_program(int(sys.argv[1]) if len(sys.argv) > 1 else S,
                       int(sys.argv[2]) if len(sys.argv) > 2 else S - 1)
    print(f"build+finalize: {time.time()-t0:.1f}s", flush=True)

